# revision 22
# baseline (speedup 1.0000x reference)
"""Trainium2 Bass kernel for nn_BondPredictor (gnn_message_passing).

Computes, for each batch b:
    A      = hidden_states[b][clip(atom_indices[b])]          # [256, 512] gather
    pair   = concat(A[i]+A[j], |A[i]-A[j]|)                   # [256,256,1024]
    h      = gelu(pair @ W1 + b1)                             # [256,256,512]
    logits = h @ W2 + b2  -> [7, 256, 256], diagonal = -10000

Sharding: 8 cores = 2 batches x 4 row-blocks of 64 rows. Each core's atom
axis is ROLLED by -64*(c%4) so every core computes rows 0..63 of its rolled
grid with an identical program (pure SPMD); the host un-rolls the output
columns when unsharding.

Symmetry: pair(i,j) == pair(j,i) exactly, so each row only computes the
cyclic half-window of columns (j-i) mod 256 in [1,128] (the diagonal is
skipped -- the host writes the -10000 fill itself -- and offset 128 rides
the shifted window, so no separate antipodal pass); the host mirrors
offsets 129..255 from the transpose during unshard. With 64 local rows the
window never wraps (63+128 < 256), so no doubled [A|A] tiles either.

Algebraic split: |ai-aj| = 2*max(ai,aj) - (ai+aj), so
    pair @ W1 = max(ai,aj) @ 2*W1b + P'[i] + P'[j]  with  P' = A @ (W1a-W1b).
Per quad of 4 rows x 128 cyclic columns, the P'[j] term enters PSUM through
an identity-weight DoublePixel matmul whose rhs is an overlapping-window
access pattern on P' (stride-1 on both the row and column dims), and
P'[i]+b1 through a second one with a stride-0 broadcast access pattern --
no per-pair rank-one materialization at all. The max features go straight
to fp8 as small DVE tensor_scalar ops (GPSIMD measured prohibitively slow
per-op on real HW, so everything element-wise stays on DVE/ScalarE); the
pair matmul runs as fp8e4 DoubleRow against x32(W1b) weights (scaling
keeps fp8 in the normal range; the gelu activation scale undoes the x16
data scale), and the 7-wide output head as fp8e4 DoubleRow with weights
padded to 16 columns (dual-fp8 ldweights ISA minimum), biased via an
Identity activation with scale 1/32. All bf16 matmuls (gather, P',
identity injections) use DoublePixel (2 rhs columns/cycle, numerics
verified exact on hardware; ~2x measured end-to-end). Exact erf-GELU on
ScalarE in two [128,1024] ops per quad. Inputs arrive in 1-2 strided DMAs
per tensor and outputs leave in 4-quad batches to keep the sync sequencer
off the critical path.
"""

import sys

sys.path.insert(0, "/opt/trn_rl_repo")

import numpy as np
import ml_dtypes

B, T, D, N, C = 2, 1024, 512, 256, 7
NCORES = 8
RB = 4                # row-blocks per batch
NL = N // RB          # 64 rows per core
QR = 4                # rows per quad
NQ = NL // QR         # 16 quads
KC = D // 128         # 4 chunks of the 512-dim contraction
TC_ = T // 128        # 8 chunks of the sequence dim
TW = 128              # cols per row: cyclic offsets (j-i) mod N in [1,128];
                      # offsets 129..255 come from the exact grid symmetry
QB = 4                # quads per output DMA batch
C8 = 16               # fp8 head column pad: dual-fp8 ldweights needs >= 16 cols
MASK_FILL = -10000.0
W1SC = 16.0           # host scale on (W1a-W1b) and b1 (undone by act scale)
W8SC = 32.0           # host scale on fp8 W2 (undone by the epilogue act scale)

_CACHE = {}


def _build(reps=1):
    """Build + compile the per-core Bass program. Returns nc."""
    import os
    import bass_rust
    import concourse.bass as bass
    import concourse.bacc as bacc
    import concourse.tile as tile
    from concourse import mybir

    f32 = mybir.dt.float32
    bf16 = mybir.dt.bfloat16
    f8 = mybir.dt.float8e4
    i32 = mybir.dt.int32
    Alu = mybir.AluOpType
    Act = mybir.ActivationFunctionType
    PM = mybir.MatmulPerfMode
    DP = PM.DoublePixel
    if os.environ.get("GELU_SUB"):
        Act_Gelu = getattr(Act, os.environ["GELU_SUB"])
    else:
        Act_Gelu = Act.Gelu

    nc = bacc.Bacc("TRN2", target_bir_lowering=False, debug=False)

    h_d = nc.dram_tensor("h", [T, D], bf16, kind="ExternalInput")
    idx_d = nc.dram_tensor("idxf", [1, N], f32, kind="ExternalInput")
    w1m_d = nc.dram_tensor("w1m", [D, D], bf16, kind="ExternalInput")
    w18a_d = nc.dram_tensor("w18a", [128, 2, D], f8, kind="ExternalInput")
    w18b_d = nc.dram_tensor("w18b", [128, 2, D], f8, kind="ExternalInput")
    w28a_d = nc.dram_tensor("w28a", [128, 2, C8], f8, kind="ExternalInput")
    w28b_d = nc.dram_tensor("w28b", [128, 2, C8], f8, kind="ExternalInput")
    b1_d = nc.dram_tensor("b1", [D, 1], f32, kind="ExternalInput")
    b2_d = nc.dram_tensor("b2", [C, 1], f32, kind="ExternalInput")
    out_d = nc.dram_tensor("out", [C, NL, TW], bf16, kind="ExternalOutput")

    idx_ap, b2_ap, out_ap = idx_d.ap(), b2_d.ap(), out_d.ap()

    def dram_ap(d, pat, off=0):
        v = d.ap()
        return bass_rust.AP(v.tensor, off, pat)

    def win_ap(tile_, off, nrow, ncol, cstride=1):
        """[128, nrow, ncol] view of a 2D tile: (r, c) -> col off + r + c*cstride."""
        v = tile_[:]
        part = v.ap.to_list()[0]
        return bass_rust.AP(v.tensor, v.offset + off, [part, [1, nrow], [cstride, ncol]])

    def dup_ap(ps, n):
        """[128, 2, n] read view duplicating the first n cols of psum tile ps."""
        v = ps[:]
        part = v.ap.to_list()[0]
        return bass_rust.AP(v.tensor, v.offset, [part, [0, 2], [1, n]])

    with tile.TileContext(nc) as tc:
        from contextlib import ExitStack

        with ExitStack() as ctx:
            const = ctx.enter_context(tc.tile_pool(name="const", bufs=1))
            wpool = ctx.enter_context(tc.tile_pool(name="w", bufs=1))
            gpool = ctx.enter_context(tc.tile_pool(name="g", bufs=3))
            fpool = ctx.enter_context(tc.tile_pool(name="f", bufs=3))
            opool = ctx.enter_context(tc.tile_pool(name="o", bufs=2))
            ph = ctx.enter_context(
                tc.tile_pool(name="ph", bufs=3, space=bass.MemorySpace.PSUM)
            )
            po = ctx.enter_context(
                tc.tile_pool(name="po", bufs=2, space=bass.MemorySpace.PSUM)
            )

            # ---- one-time constants (outside rep loop) ----
            ones1 = const.tile([1, 128], f32, tag="ones1")
            nc.vector.memset(ones1[:], 1.0)
            onesq = const.tile([128, 128], bf16, tag="onesq")
            nc.vector.memset(onesq[:], 1.0)
            ident = const.tile([128, 128], bf16, tag="ident")
            # iota[p,f] = p - f -> ==0 on the diagonal
            nc.gpsimd.affine_select(
                ident[:], onesq[:], pattern=[[-1, 128]],
                compare_op=Alu.is_equal, fill=0.0, base=0, channel_multiplier=1,
            )
            iota_i = const.tile([128, TC_], i32, tag="iota_i")
            nc.gpsimd.iota(iota_i[:], pattern=[[128, TC_]], base=0, channel_multiplier=1)
            iota_f = const.tile([128, TC_], f32, tag="iota_f")
            nc.vector.tensor_copy(iota_f[:], iota_i[:])

            # b1 x16 as [128, KC]: (p, m) <- b1[128m + p]
            b1c = const.tile([128, KC], f32, tag="b1c")
            nc.sync.dma_start(b1c[:], dram_ap(b1_d, [[1, 128], [128, KC]]))
            b2c = const.tile([C, 1], f32, tag="b2c")
            nc.sync.dma_start(b2c[:], b2_ap[:])

            # weights: x16(W1a-W1b) [128, KC, D] bf16; x32(2*W1b) fp8
            # DoubleRow packs; W2 [128, KC, C] bf16
            w1sb = wpool.tile([128, KC, D], bf16, tag="w1m")
            nc.sync.dma_start(
                w1sb[:], dram_ap(w1m_d, [[D, 128], [128 * D, KC], [1, D]])
            )
            w18 = []
            for p, d_ in enumerate([w18a_d, w18b_d]):
                t = wpool.tile([128, 2, D], f8, tag=f"w18_{p}")
                nc.sync.dma_start(t[:], d_.ap())
                w18.append(t)
            w28 = []
            for p, d_ in enumerate([w28a_d, w28b_d]):
                t = wpool.tile([128, 2, C8], f8, tag=f"w28_{p}")
                nc.sync.dma_start(t[:], d_.ap())
                w28.append(t)

            def body():
                # ---- inputs: idx first (phase A gate), then h in one DMA ----
                idx_sb = gpool.tile([1, N], f32, tag="idx_sb")
                nc.sync.dma_start(idx_sb[:], idx_ap[:])
                ht = gpool.tile([128, TC_, D], bf16, tag="ht")
                nc.sync.dma_start(
                    ht[:], dram_ap(h_d, [[D, 128], [128 * D, TC_], [1, D]])
                )

                # ---- phase A: one-hot of idx, [t, n] layout ----
                ps_i = po.tile([128, N], f32, tag="po")
                nc.tensor.matmul(ps_i[:], ones1[:], idx_sb[:])  # bcast idx to 128 parts
                idxb = gpool.tile([128, N], f32, tag="idxb")
                nc.vector.tensor_copy(idxb[:], ps_i[:])
                oh = []
                for t_ in range(TC_):
                    o = gpool.tile([128, N], bf16, tag=f"oh_{t_}")
                    nc.vector.tensor_scalar(
                        o[:], idxb[:], iota_f[:, t_ : t_ + 1], None, op0=Alu.is_equal
                    )
                    oh.append(o)

                # ---- phase B: gather A_T = H_T @ onehot; doubled [A|A] +
                # f32 row-scalar copies ----
                at2_bf, at32 = [], []
                for m in range(KC):
                    ps_g = po.tile([128, N], f32, tag="po")
                    for t_ in range(TC_):
                        nc.tensor.matmul(
                            ps_g[:],
                            ht[:, t_, 128 * m : 128 * (m + 1)],
                            oh[t_][:],
                            start=(t_ == 0),
                            stop=(t_ == TC_ - 1),
                            perf_mode=DP,
                        )
                    a_bf = gpool.tile([128, N], bf16, tag=f"at_{m}")
                    nc.vector.tensor_copy(a_bf[:], ps_g[:])
                    a32 = gpool.tile([128, NL], f32, tag=f"at32_{m}")
                    nc.vector.tensor_copy(a32[:], ps_g[:, 0:NL])
                    at2_bf.append(a_bf)
                    at32.append(a32)

                # ---- phase C: P' = A @ x16(W1a-W1b), doubled + biased copies ----
                pq2x, ppb = [], []
                for m in range(KC):
                    ps_p = po.tile([128, N], f32, tag="po")
                    for k in range(KC):
                        nc.tensor.matmul(
                            ps_p[:],
                            w1sb[:, k, 128 * m : 128 * (m + 1)],
                            at2_bf[k][:],
                            start=(k == 0),
                            stop=(k == KC - 1),
                            perf_mode=DP,
                        )
                    p2 = gpool.tile([128, N], bf16, tag=f"pq_{m}")
                    nc.vector.tensor_copy(p2[:], ps_p[:])
                    pb = gpool.tile([128, NL], bf16, tag=f"ppb_{m}")
                    nc.vector.tensor_scalar(
                        pb[:], ps_p[:, 0:NL], b1c[:, m : m + 1], None, op0=Alu.add
                    )
                    pq2x.append(p2)
                    ppb.append(pb)

                # ---- main loop over row-quads: row i covers cyclic cols
                # j = i..i+127 (the symmetric half of the grid) ----
                outg = None
                for q in range(NQ):
                    i0 = QR * q
                    g = q % QB
                    if g == 0:
                        outg = opool.tile([C, QB, QR * TW], bf16, tag="outg")
                    # max(ai, aj) features, fp8, [128, k, (r, c)] layout
                    # (all on DVE: real-HW GPSIMD per-op cost is prohibitive)
                    mxq = fpool.tile([128, KC, QR * TW], f8, tag="mxq")
                    for k in range(KC):
                        for r in range(QR):
                            i = i0 + r
                            nc.vector.tensor_scalar(
                                mxq[:, k, TW * r : TW * (r + 1)],
                                at2_bf[k][:, i + 1 : i + 1 + TW],
                                at32[k][:, i : i + 1],
                                None,
                                op0=Alu.max,
                            )

                    hh = fpool.tile([128, KC, QR * TW], f8, tag="hh")
                    for half in range(2):
                        ps_h = ph.tile([128, 1024], f32, tag="ph")
                        for mh in range(2):
                            m = 2 * half + mh
                            sl = ps_h[:, 512 * mh : 512 * (mh + 1)]
                            nc.tensor.matmul(
                                sl, ident[:], win_ap(pq2x[m], i0 + 1, QR, TW),
                                start=True, stop=False, perf_mode=DP,
                            )
                            nc.tensor.matmul(
                                sl, ident[:], win_ap(ppb[m], i0, QR, TW, cstride=0),
                                start=False, stop=False, perf_mode=DP,
                            )
                            for p in range(2):
                                nc.tensor.matmul(
                                    sl,
                                    w18[p][:, :, 128 * m : 128 * (m + 1)],
                                    mxq[:, 2 * p : 2 * p + 2, :],
                                    start=False,
                                    stop=(p == 1),
                                    perf_mode=PM.DoubleRow,
                                )
                        nc.scalar.activation(
                            hh[:, 2 * half : 2 * half + 2, :], ps_h[:],
                            Act_Gelu, scale=1.0 / W1SC,
                        )

                    ps_o = po.tile([C8, QR * TW], f32, tag="po")
                    for p in range(2):
                        nc.tensor.matmul(
                            ps_o[:], w28[p][:], hh[:, 2 * p : 2 * p + 2, :],
                            start=(p == 0), stop=(p == 1),
                            perf_mode=PM.DoubleRow,
                        )
                    nc.scalar.activation(
                        outg[:, g, :], ps_o[0:C, :], Act.Identity,
                        bias=b2c[:], scale=1.0 / W8SC,
                    )
                    if g == QB - 1:
                        g0 = QR * (q - QB + 1)
                        nc.sync.dma_start(
                            out_ap[:, g0 : g0 + QB * QR, 0:TW], outg[:]
                        )

            for _ in range(reps):
                body()

    nc.compile()
    return nc


def _get(reps=1):
    if reps not in _CACHE:
        _CACHE[reps] = _build(reps)
    return _CACHE[reps]


def _shard_inputs(hidden_states, W1, b1, W2, b2, atom_indices):
    hs = np.asarray(hidden_states, np.float32)
    idx = np.clip(np.asarray(atom_indices).astype(np.int64), 0, T - 1)
    w1 = np.asarray(W1, np.float32)
    w1m = (W1SC * (w1[0:D] - w1[D : 2 * D])).astype(ml_dtypes.bfloat16)
    w1b = W1SC * 2.0 * w1[D : 2 * D]          # [512, 512] k x m; /W1SC in act scale
    f8 = ml_dtypes.float8_e4m3

    def pack_pairs(mat, p):  # mat [512, cols] -> [128, 2, cols] chunks 2p, 2p+1
        return np.stack(
            [mat[128 * (2 * p + i) : 128 * (2 * p + i) + 128] for i in range(2)],
            axis=1,
        ).astype(f8)

    w18a, w18b = pack_pairs(w1b, 0), pack_pairs(w1b, 1)
    w2s = np.zeros((D, C8), np.float32)       # [512, 16], cols 7.. zero-padded
    w2s[:, 0:C] = W8SC * np.asarray(W2, np.float32)
    w28a, w28b = pack_pairs(w2s, 0), pack_pairs(w2s, 1)
    b1f = (W1SC * np.asarray(b1, np.float32)).reshape(D, 1)
    b2f = np.asarray(b2, np.float32).reshape(C, 1)
    in_maps = []
    for c in range(NCORES):
        b = c // RB
        r0 = NL * (c % RB)
        idx_roll = np.roll(idx[b], -r0).astype(np.float32).reshape(1, N)
        in_maps.append(
            {
                "h": hs[b].astype(ml_dtypes.bfloat16),
                "idxf": idx_roll,
                "w1m": w1m,
                "w18a": w18a,
                "w18b": w18b,
                "w28a": w28a,
                "w28b": w28b,
                "b1": b1f,
                "b2": b2f,
            }
        )
    return in_maps


def _unshard(results, atom_mask):
    full = np.zeros((B, C, N, N), np.float32)
    for c in range(NCORES):
        b = c // RB
        r0 = NL * (c % RB)
        blk = np.asarray(results[c]["out"]).astype(np.float32)  # [C, 64, 128]
        rows = r0 + np.arange(NL)
        idx_j = (rows[:, None] + 1 + np.arange(TW)[None, :]) % N  # [64, 128]
        np.put_along_axis(
            full[b, :, r0 : r0 + NL, :],
            np.broadcast_to(idx_j[None], (C, NL, TW)),
            blk,
            axis=2,
        )
    # grid symmetry: logits[i,j] == logits[j,i]; offsets 129..255 mirror
    offs = (np.arange(N)[None, :] - np.arange(N)[:, None]) % N
    low = offs > TW
    fullT = np.transpose(full, (0, 1, 3, 2))
    full = np.where(low[None, None], fullT, full)
    # diagonal + mask fill on host (device leaves true logits there)
    di = np.arange(N)
    full[:, :, di, di] = MASK_FILL
    mask = np.asarray(atom_mask).astype(bool)
    if not mask.all():
        valid = mask[:, :, None] & mask[:, None, :]
        valid &= ~np.eye(N, dtype=bool)[None]
        full = np.where(valid[:, None, :, :], full, np.float32(MASK_FILL))
    return full


def kernel(hidden_states, W1, b1, W2, b2, atom_indices, atom_mask):
    from concourse.bass_utils import run_bass_kernel_spmd

    nc = _get(1)
    in_maps = _shard_inputs(hidden_states, W1, b1, W2, b2, atom_indices)
    res = run_bass_kernel_spmd(nc, in_maps, list(range(NCORES)))
    return _unshard(res.results, atom_mask)
